# revision 1
# baseline (speedup 1.0000x reference)
"""CREStereo deformable local correlation on 8 Trainium2 NeuronCores.

Sharding: data-parallel over (batch 2) x (H quarters 4) = 8 cores.
Per core: bilinear-sample right_feature (channel-last in DRAM) at
data-dependent coords via SWDGE dma_gather (1KB/pixel elements), blend the
4 corners + multiply by left + group-reduce on DVE/GPSIMD.
All arithmetic (coords, floor, weights, masks, indices) happens on device;
the host only slices/transposes/pads (layout) and concatenates shards.
"""
import sys
sys.path.insert(0, "/opt/trn_rl_repo")
import numpy as np

import concourse.bass as bass
import concourse.bacc as bacc
import concourse.mybir as mybir
import concourse.tile as tile
from concourse import bass_utils
from concourse.library_config import mlp

B, C, H, W = 2, 256, 96, 192
K, G, GC = 9, 4, 64
HQ = H // 4            # 24 rows per shard
HALO = 12
ROWS = HQ + 2 * HALO   # 48
NPIX = HQ * W          # 4608 pixels per shard
NT = NPIX // 128       # 36 tiles of 128 pixels
NSRC = ROWS * W        # 9216 gatherable pixel rows
NTT = NT * K           # 324 sample tiles (T = t'*9 + k)
NIDX = NTT * 4 * 128   # 165888 gather indices
CHUNK_T = 2            # T-tiles per dma_gather (1024 idxs, ring limit ~2032)
F32 = mybir.dt.float32
I16 = mybir.dt.int16

_cache = {}


def _build():
    if "nc" in _cache:
        return _cache["nc"]
    nc = bacc.Bacc("TRN2", debug=False, num_devices=8, num_swdge_queues=4)
    right_t = nc.dram_tensor("right_cl", [NSRC, C], F32, kind="ExternalInput")
    left_t = nc.dram_tensor("left_cl", [NPIX, C], F32, kind="ExternalInput")
    flow_t = nc.dram_tensor("flow_t", [2, 128, NT], F32, kind="ExternalInput")
    extra_t = nc.dram_tensor("extra_t", [2, 128, NT, K], F32, kind="ExternalInput")
    hglob_t = nc.dram_tensor("hglob", [128, NT], F32, kind="ExternalInput")
    wk_t = nc.dram_tensor("wk", [128, NT, K], F32, kind="ExternalInput")
    rowp_t = nc.dram_tensor("rowp", [128, NT], F32, kind="ExternalInput")
    out_t = nc.dram_tensor("out", [NPIX, G * K], F32, kind="ExternalOutput")

    KT = K * NT  # 324
    with tile.TileContext(nc) as tc:
        with tc.tile_pool(name="const", bufs=1) as cpool, \
             tc.tile_pool(name="left", bufs=1) as lpool, \
             tc.tile_pool(name="math", bufs=1) as mpool, \
             tc.tile_pool(name="idx", bufs=1) as ipool, \
             tc.tile_pool(name="g", bufs=8) as gpool, \
             tc.tile_pool(name="blend", bufs=4) as bpool, \
             tc.tile_pool(name="oacc", bufs=4) as opool:
            nc.gpsimd.load_library(mlp)

            # ---- load small tensors ----
            flow = cpool.tile([128, 2, NT], F32)
            nc.sync.dma_start(flow[:], flow_t.ap().rearrange("c p t -> p c t"))
            extra = cpool.tile([128, 2, KT], F32)
            nc.sync.dma_start(extra[:], extra_t.ap().rearrange("c p t k -> p c (t k)"))
            hglob = cpool.tile([128, NT], F32)
            nc.sync.dma_start(hglob[:], hglob_t.ap())
            wk = cpool.tile([128, KT], F32)
            nc.sync.dma_start(wk[:], wk_t.ap())
            rowp = cpool.tile([128, NT], F32)
            nc.sync.dma_start(rowp[:], rowp_t.ap())
            left = lpool.tile([128, NT, C], F32)
            nc.sync.dma_start(left[:], left_t.ap().rearrange("(t p) c -> p t c", p=128))

            def bc_k(ap2d):  # [128, NT] -> [128, NT, K(bcast)]
                return ap2d.rearrange("p (t o) -> p t o", o=1).broadcast_to([128, NT, K])

            # ---- index & weight math, [128, K*NT] f32 ----
            AF = mybir.AluOpType
            xq = mpool.tile([128, NT, K], F32)
            yq = mpool.tile([128, NT, K], F32)
            nc.vector.tensor_tensor(xq[:], extra[:, 0].rearrange("p (t k) -> p t k", k=K),
                                    bc_k(flow[:, 0]), op=AF.add)
            nc.vector.tensor_tensor(xq[:], xq[:], wk[:].rearrange("p (t k) -> p t k", k=K),
                                    op=AF.add)
            nc.vector.tensor_tensor(yq[:], extra[:, 1].rearrange("p (t k) -> p t k", k=K),
                                    bc_k(flow[:, 1]), op=AF.add)
            nc.vector.tensor_tensor(yq[:], yq[:], bc_k(hglob[:]), op=AF.add)
            # biased coords (positive -> trunc == floor)
            xb = mpool.tile([128, KT], F32)
            yb = mpool.tile([128, KT], F32)
            nc.vector.tensor_scalar(xb[:], xq[:].rearrange("p t k -> p (t k)"), 64.0, None, op0=AF.add)
            nc.vector.tensor_scalar(yb[:], yq[:].rearrange("p t k -> p (t k)"), 64.0, None, op0=AF.add)
            x0i = mpool.tile([128, KT], I16)
            y0i = mpool.tile([128, KT], I16)
            nc.vector.tensor_copy(x0i[:], xb[:])
            nc.vector.tensor_copy(y0i[:], yb[:])
            x0f = mpool.tile([128, KT], F32)
            y0f = mpool.tile([128, KT], F32)
            nc.vector.tensor_copy(x0f[:], x0i[:])
            nc.vector.tensor_copy(y0f[:], y0i[:])
            # cast rounding mode differs sim vs hw; force floor: t -= (t > x)
            gfix = mpool.tile([128, KT], F32, tag="gfix")
            nc.vector.tensor_tensor(gfix[:], x0f[:], xb[:], op=AF.is_gt)
            nc.vector.tensor_tensor(x0f[:], x0f[:], gfix[:], op=AF.subtract)
            nc.vector.tensor_copy(x0i[:], x0f[:])
            nc.vector.tensor_tensor(gfix[:], y0f[:], yb[:], op=AF.is_gt)
            nc.vector.tensor_tensor(y0f[:], y0f[:], gfix[:], op=AF.subtract)
            nc.vector.tensor_copy(y0i[:], y0f[:])
            wx1 = mpool.tile([128, KT], F32)
            wy1 = mpool.tile([128, KT], F32)
            nc.vector.tensor_tensor(wx1[:], xb[:], x0f[:], op=AF.subtract)
            nc.vector.tensor_tensor(wy1[:], yb[:], y0f[:], op=AF.subtract)
            wx0 = mpool.tile([128, KT], F32)
            wy0 = mpool.tile([128, KT], F32)
            # 1 - w = (w - 1) * -1
            nc.vector.tensor_scalar(wx0[:], wx1[:], 1.0, -1.0, op0=AF.subtract, op1=AF.mult)
            nc.vector.tensor_scalar(wy0[:], wy1[:], 1.0, -1.0, op0=AF.subtract, op1=AF.mult)
            # validity (biased domain: valid x0 in [64, 64+W-1], x1=x0+1 in range -> x0 in [63, 64+W-2])
            def valid(dst, src, lo, hi, scale):
                t1 = mpool.tile([128, KT], F32, tag="vtmp1")
                nc.vector.tensor_scalar(t1[:], src[:], float(lo), float(scale),
                                        op0=AF.is_ge, op1=AF.mult)
                t2 = mpool.tile([128, KT], F32, tag="vtmp2")
                nc.vector.tensor_scalar(t2[:], src[:], float(hi), None, op0=AF.is_le)
                nc.vector.tensor_tensor(dst[:], t1[:], t2[:], op=AF.mult)
            vx0 = mpool.tile([128, KT], F32)
            vx1 = mpool.tile([128, KT], F32)
            vy0 = mpool.tile([128, KT], F32)
            vy1 = mpool.tile([128, KT], F32)
            valid(vx0, x0f, 64, 64 + W - 1, 1.0)
            valid(vx1, x0f, 63, 64 + W - 2, 1.0)
            valid(vy0, y0f, 64, 64 + H - 1, 1.0 / GC)   # fold 1/64 group mean
            valid(vy1, y0f, 63, 64 + H - 2, 1.0 / GC)
            wxv0, wxv1 = vx0, vx1   # reuse in place
            nc.vector.tensor_tensor(wxv0[:], wx0[:], vx0[:], op=AF.mult)
            nc.vector.tensor_tensor(wxv1[:], wx1[:], vx1[:], op=AF.mult)
            wyv0, wyv1 = vy0, vy1
            nc.vector.tensor_tensor(wyv0[:], wy0[:], vy0[:], op=AF.mult)
            nc.vector.tensor_tensor(wyv1[:], wy1[:], vy1[:], op=AF.mult)
            wt4 = mpool.tile([128, 4, KT], F32)
            nc.vector.tensor_tensor(wt4[:, 0], wyv0[:], wxv0[:], op=AF.mult)
            nc.vector.tensor_tensor(wt4[:, 1], wyv0[:], wxv1[:], op=AF.mult)
            nc.vector.tensor_tensor(wt4[:, 2], wyv1[:], wxv0[:], op=AF.mult)
            nc.vector.tensor_tensor(wt4[:, 3], wyv1[:], wxv1[:], op=AF.mult)
            # ---- flat gather indices (int16) ----
            # idx00 = (y0 - 64 + HALO)*W + (x0 - 64), y0/x0 biased ints
            idx4 = ipool.tile([128, 4, KT], I16)
            # f32 index base: ((y0f - row0)*W + x0f + ((HALO-64)*W - 64)), exact ints in f32
            idxf = mpool.tile([128, KT], F32, tag="idxf")
            nc.vector.tensor_tensor(idxf[:].rearrange("p (t k) -> p t k", k=K),
                                    y0f[:].rearrange("p (t k) -> p t k", k=K),
                                    bc_k(rowp[:]), op=AF.subtract)
            nc.vector.tensor_scalar(idxf[:], idxf[:], float(W), float((HALO - 64) * W - 64),
                                    op0=AF.mult, op1=AF.add)
            nc.vector.tensor_tensor(idxf[:], idxf[:], x0f[:], op=AF.add)
            nc.vector.tensor_copy(idx4[:, 0], idxf[:])
            nc.vector.tensor_scalar(idx4[:, 1], idx4[:, 0], 1, None, op0=AF.add)
            nc.vector.tensor_scalar(idx4[:, 2], idx4[:, 0], W, None, op0=AF.add)
            nc.vector.tensor_scalar(idx4[:, 3], idx4[:, 0], W + 1, None, op0=AF.add)
            nc.vector.tensor_scalar(idx4[:].rearrange("p j f -> p (j f)"),
                                    idx4[:].rearrange("p j f -> p (j f)"), 0, NSRC - 1,
                                    op0=AF.max, op1=AF.min)
            # ---- rearrange idx to wrapped layout [16, NIDX/16] replicated x8 ----
            # stream pos i = ((t*K + k)*4 + j)*128 + lane ; wrapped [i%16, i//16]
            # i//16 = t*(K*4*8) + k*32 + j*8 + lh  where lane = lh*16 + p16
            wrapped = ipool.tile([128, NIDX // 16], I16)
            wdst = wrapped[:].rearrange("p (tk j l) -> p tk j l", tk=NTT, j=4)
            for lh in range(8):
                for jj in range(4):
                    # src partitions lh*16..+16 ; (t,k) contiguous on both sides
                    nc.sync.dma_start(
                        wdst[0:16, :, jj, lh],
                        idx4[lh * 16:(lh + 1) * 16, jj])
            for cc in range(1, 8):
                nc.sync.dma_start(wrapped[cc * 16:(cc + 1) * 16, :], wrapped[0:16, :])

            # ---- gather + blend loop ----
            oaccs = {}
            NCHUNK = NTT // CHUNK_T
            for ci in range(NCHUNK):
                gt = gpool.tile([128, CHUNK_T * 4, C], F32, tag="gt")
                nc.gpsimd.dma_gather(
                    gt[:], right_t.ap(),
                    wrapped[:, ci * (CHUNK_T * 4 * 8):(ci + 1) * (CHUNK_T * 4 * 8)],
                    CHUNK_T * 4 * 128, CHUNK_T * 4 * 128, C, queue_num=ci % 4)
                for st in range(CHUNK_T):
                    T = ci * CHUNK_T + st
                    t_, k_ = T // K, T % K  # t-major stream
                    if k_ == 0:
                        oaccs[t_] = opool.tile([128, G * K], F32, tag="oacc", name=f"oacc{t_}")
                    oacc = oaccs[t_]
                    gsl = gt[:, st * 4:(st + 1) * 4]            # [128, 4, C]
                    wsl = wt4[:, :, T]                          # [128, 4]
                    tmp = bpool.tile([128, 4, C], F32, tag="tmp")
                    nc.gpsimd.tensor_tensor(
                        tmp[:], gsl,
                        wsl.rearrange("p (o j) -> p j o", o=1).broadcast_to([128, 4, C]),
                        op=AF.mult)
                    s = bpool.tile([128, C], F32, tag="s")
                    nc.vector.tensor_reduce(
                        s[:], tmp[:].rearrange("p j c -> p c j"),
                        axis=mybir.AxisListType.X, op=AF.add)
                    nc.vector.tensor_tensor(s[:], s[:], left[:, t_], op=AF.mult)
                    nc.vector.tensor_reduce(
                        oacc[:].rearrange("p (g k) -> p g k", g=G)[:, :, k_],
                        s[:].rearrange("p (g c) -> p g c", g=G),
                        axis=mybir.AxisListType.X, op=AF.add)
                    if k_ == K - 1:
                        nc.sync.dma_start(
                            out_t.ap().rearrange("(t p) f -> p t f", p=128)[:, t_], oacc[:])
    nc.compile()
    _cache["nc"] = nc
    return nc


def make_in_maps(left_feature, right_feature, flow, extra_offset):
    left_feature = np.ascontiguousarray(np.asarray(left_feature), dtype=np.float32)
    right_feature = np.ascontiguousarray(np.asarray(right_feature), dtype=np.float32)
    flow = np.asarray(flow, dtype=np.float32)
    extra_offset = np.asarray(extra_offset, dtype=np.float32)

    pix = np.arange(NPIX)
    lane, tt = pix % 128, pix // 128

    def plane(vals):
        p = np.zeros((128, NT), np.float32)
        p[lane, tt] = vals
        return p

    kgrid = np.arange(-4, 5, dtype=np.float32)
    in_maps = []
    for b in range(B):
        l_cl = np.ascontiguousarray(left_feature[b].transpose(1, 2, 0))
        r_cl = np.ascontiguousarray(right_feature[b].transpose(1, 2, 0))
        eo = extra_offset[b].reshape(K, 2, H, W)
        for q in range(4):
            row0 = q * HQ
            lo, hi = row0 - HALO, row0 + HQ + HALO
            r_slice = np.zeros((ROWS, W, C), np.float32)
            clo, chi = max(lo, 0), min(hi, H)
            r_slice[clo - lo:chi - lo] = r_cl[clo:chi]
            hgl = plane((pix // W + row0).astype(np.float32))
            rwp = np.full((128, KK_NT), np.float32(row0)) if False else np.full((128, NT), np.float32(row0), np.float32)
            wgr = (pix % W).astype(np.float32)
            wkp = np.stack([plane(wgr + kg) for kg in kgrid], axis=1)  # [128, K, NT]
            wkp = np.ascontiguousarray(wkp.transpose(0, 2, 1))          # [128, NT, K]
            fl = np.stack([plane(flow[b, c_][row0:row0 + HQ].ravel()) for c_ in range(2)])
            ex = np.stack([np.stack([plane(eo[k_, c_, row0:row0 + HQ].ravel())
                                     for k_ in range(K)], axis=1) for c_ in range(2)])
            ex = np.ascontiguousarray(ex.transpose(0, 1, 3, 2))         # [2, 128, NT, K]
            in_maps.append({
                "right_cl": np.ascontiguousarray(r_slice.reshape(NSRC, C)),
                "left_cl": np.ascontiguousarray(l_cl[row0:row0 + HQ].reshape(NPIX, C)),
                "flow_t": np.ascontiguousarray(fl),     # [2, 128, NT]
                "extra_t": np.ascontiguousarray(ex),    # [2, 128, K, NT]
                "hglob": hgl,
                "wk": wkp,
                "rowp": rwp,
            })
    return in_maps


def assemble(results):
    out = np.zeros((B, G * K, H, W), np.float32)
    for b in range(B):
        for q in range(4):
            o = results[b * 4 + q]["out"]              # [NPIX, G*K], row = pixel
            out[b, :, q * HQ:(q + 1) * HQ] = o.T.reshape(G * K, HQ, W)
    return out


def kernel(left_feature, right_feature, flow, extra_offset):
    nc = _build()
    in_maps = make_in_maps(left_feature, right_feature, flow, extra_offset)
    res = bass_utils.run_bass_kernel_spmd(nc, in_maps, list(range(8)))
    return assemble(res.results)



# revision 30
# speedup vs baseline: 2.1631x; 2.1631x over previous
"""CREStereo deformable local correlation on 8 Trainium2 NeuronCores.

Sharding: data-parallel over (batch 2) x (H quarters 4) = 8 cores.

Per core (v2, channel-major matmul pipeline):
  - SWDGE transpose-gather of fp16 x-pair feature elements (2 pixels x 256ch
    = 1KB) into channel-major SBUF tiles: out[c, (x,ch), i].
  - DVE multiplies gathered features by left (channel-major, pre-scaled 1/64)
    in fp16 at 2x rate.
  - TensorE contracts the 64-channel groups with ones-block stationaries
    (r-replica trick spreads 128 pixel-lanes x 4 groups x 4 corners over
    PSUM partitions), accumulating the two 128-channel halves in PSUM.
  - Corner (bilinear) weights apply post-contraction at scalar level on DVE,
    followed by a 4-corner reduce; all index/weight math runs on device.
All value arithmetic happens on device; the host only slices/transposes/pads
(layout) and concatenates shards.
"""
import sys
sys.path.insert(0, "/opt/trn_rl_repo")
import numpy as np

import concourse.bass as bass
import concourse.bacc as bacc
import concourse.mybir as mybir
import concourse.tile as tile
from concourse.ap import AP
from concourse import bass_utils
from concourse.library_config import mlp

B, C, H, W = 2, 256, 96, 192
K, G, GC = 9, 4, 64
HQ = H // 4            # 24 rows per shard
HALO = 12
ROWS = HQ + 2 * HALO   # 48
NPIX = HQ * W          # 4608 pixels per shard
NT = NPIX // 128       # 36 tiles of 128 pixels
NSRC = ROWS * W        # 9216 gatherable pixel rows
T = NT * K             # 324 (t,k) tiles
NI_T = K * 2 * 128     # 2304 gather idxs per t (k, yc, lane)
F32 = mybir.dt.float32
FP16 = mybir.dt.float16
I16 = mybir.dt.int16

_cache = {}


def _build():
    if "nc" in _cache:
        return _cache["nc"]
    nc = bacc.Bacc("TRN2", debug=False, num_devices=8, num_swdge_queues=4)
    right_t = nc.dram_tensor("right_cl", [NSRC + 1, C], FP16, kind="ExternalInput")
    left_t = nc.dram_tensor("left_cm", [2, 128, NPIX], FP16, kind="ExternalInput")
    flow_t = nc.dram_tensor("flow_t", [2, 128, NT], F32, kind="ExternalInput")
    extra_t = nc.dram_tensor("extra_t", [2, 128, NT, K], F32, kind="ExternalInput")
    hglob_t = nc.dram_tensor("hglob", [128, NT], F32, kind="ExternalInput")
    wk_t = nc.dram_tensor("wk", [128, NT, K], F32, kind="ExternalInput")
    rowp_t = nc.dram_tensor("rowp", [128, NT], F32, kind="ExternalInput")
    out_t = nc.dram_tensor("out", [128, 4, NT, K], F32, kind="ExternalOutput")

    AF = mybir.AluOpType
    with tile.TileContext(nc) as tc:
        with tc.tile_pool(name="persist", bufs=1) as pp, \
             tc.tile_pool(name="gather", bufs=2) as gpool, \
             tc.tile_pool(name="prod", bufs=2) as prpool, \
             tc.psum_pool(name="ps", bufs=4) as pspool:
            nc.gpsimd.load_library(mlp)

            # ---- persistent tiles ----
            left = pp.tile([128, 2, NPIX], FP16)
            nc.sync.dma_start(left[:], left_t.ap().rearrange("h c p -> c h p"))
            # ones-pattern for stationary windows
            P = pp.tile([128, 256], FP16)
            nc.vector.memset(P[:], 0)
            nc.vector.memset(P[0:64, 128:129], 1.0)
            nc.vector.memset(P[64:128, 160:161], 1.0)
            wrapped = pp.tile([128, NT, K, 2, 8], I16)
            # drained group-dots: [(g, r32), j, ls, t, k], lane = r32*4 + ls
            drained = pp.tile([128, 4, 4, NT, K], FP16)
            w4dr = pp.tile([128, 4, 4, NT, K], FP16)  # [(g,r32), j, ls, t, k]
            finals = pp.tile([128, 4, NT, K], F32)    # [(g,r32), ls, t, k]

            # ---- index & weight math (freed after this block) ----
            with tc.tile_pool(name="math", bufs=1) as mp:
                flow = mp.tile([128, 2, NT], F32)
                nc.sync.dma_start(flow[:], flow_t.ap().rearrange("c p t -> p c t"))
                extra = mp.tile([128, 2, T], F32)
                nc.sync.dma_start(extra[:], extra_t.ap().rearrange("c p t k -> p c (t k)"))
                hglob = mp.tile([128, NT], F32)
                nc.sync.dma_start(hglob[:], hglob_t.ap())
                wk = mp.tile([128, T], F32)
                nc.sync.dma_start(wk[:], wk_t.ap().rearrange("p t k -> p (t k)"))
                rowp = mp.tile([128, NT], F32)
                nc.sync.dma_start(rowp[:], rowp_t.ap())

                def bc_k(ap2d):  # [128, NT] -> [128, NT, K(bcast)]
                    return ap2d.rearrange("p (t o) -> p t o", o=1).broadcast_to([128, NT, K])

                xq = mp.tile([128, NT, K], F32)
                yq = mp.tile([128, NT, K], F32)
                nc.vector.tensor_tensor(xq[:], extra[:, 0].rearrange("p (t k) -> p t k", k=K),
                                        bc_k(flow[:, 0]), op=AF.add)
                nc.vector.tensor_tensor(xq[:], xq[:], wk[:].rearrange("p (t k) -> p t k", k=K),
                                        op=AF.add)
                nc.vector.tensor_tensor(yq[:], extra[:, 1].rearrange("p (t k) -> p t k", k=K),
                                        bc_k(flow[:, 1]), op=AF.add)
                nc.vector.tensor_tensor(yq[:], yq[:], bc_k(hglob[:]), op=AF.add)
                # biased coords (positive -> trunc == floor)
                xb = mp.tile([128, T], F32)
                yb = mp.tile([128, T], F32)
                nc.vector.tensor_scalar(xb[:], xq[:].rearrange("p t k -> p (t k)"), 64.0, None, op0=AF.add)
                nc.vector.tensor_scalar(yb[:], yq[:].rearrange("p t k -> p (t k)"), 64.0, None, op0=AF.add)
                x0i = mp.tile([128, T], I16)
                y0i = mp.tile([128, T], I16)
                nc.vector.tensor_copy(x0i[:], xb[:])
                nc.vector.tensor_copy(y0i[:], yb[:])
                x0f = mp.tile([128, T], F32)
                y0f = mp.tile([128, T], F32)
                nc.vector.tensor_copy(x0f[:], x0i[:])
                nc.vector.tensor_copy(y0f[:], y0i[:])
                # cast rounding mode differs sim vs hw; force floor: t -= (t > x)
                gfix = mp.tile([128, T], F32, tag="gfix")
                nc.vector.tensor_tensor(gfix[:], x0f[:], xb[:], op=AF.is_gt)
                nc.vector.tensor_tensor(x0f[:], x0f[:], gfix[:], op=AF.subtract)
                nc.vector.tensor_tensor(gfix[:], y0f[:], yb[:], op=AF.is_gt)
                nc.vector.tensor_tensor(y0f[:], y0f[:], gfix[:], op=AF.subtract)
                wx1 = mp.tile([128, T], F32)
                wy1 = mp.tile([128, T], F32)
                nc.vector.tensor_tensor(wx1[:], xb[:], x0f[:], op=AF.subtract)
                nc.vector.tensor_tensor(wy1[:], yb[:], y0f[:], op=AF.subtract)
                wx0 = mp.tile([128, T], F32)
                wy0 = mp.tile([128, T], F32)
                # 1 - w = (w - 1) * -1
                nc.vector.tensor_scalar(wx0[:], wx1[:], 1.0, -1.0, op0=AF.subtract, op1=AF.mult)
                nc.vector.tensor_scalar(wy0[:], wy1[:], 1.0, -1.0, op0=AF.subtract, op1=AF.mult)

                def valid(dst, src, lo, hi):
                    t1 = mp.tile([128, T], F32, tag="vtmp1")
                    nc.vector.tensor_scalar(t1[:], src[:], float(lo), None, op0=AF.is_ge)
                    t2 = mp.tile([128, T], F32, tag="vtmp2")
                    nc.vector.tensor_scalar(t2[:], src[:], float(hi), None, op0=AF.is_le)
                    nc.vector.tensor_tensor(dst[:], t1[:], t2[:], op=AF.mult)
                vx0 = mp.tile([128, T], F32)
                vx1 = mp.tile([128, T], F32)
                vy0 = mp.tile([128, T], F32)
                vy1 = mp.tile([128, T], F32)
                valid(vx0, x0f, 64, 64 + W - 1)
                valid(vx1, x0f, 63, 64 + W - 2)
                valid(vy0, y0f, 64, 64 + H - 1)
                valid(vy1, y0f, 63, 64 + H - 2)
                wxv0, wxv1 = vx0, vx1   # reuse in place
                nc.vector.tensor_tensor(wxv0[:], wx0[:], vx0[:], op=AF.mult)
                nc.vector.tensor_tensor(wxv1[:], wx1[:], vx1[:], op=AF.mult)
                wyv0, wyv1 = vy0, vy1
                nc.vector.tensor_tensor(wyv0[:], wy0[:], vy0[:], op=AF.mult)
                nc.vector.tensor_tensor(wyv1[:], wy1[:], vy1[:], op=AF.mult)
                # corner weights, j = x*2 + yc
                wt4 = mp.tile([128, 4, T], F32)
                nc.vector.tensor_tensor(wt4[:, 0], wyv0[:], wxv0[:], op=AF.mult)
                nc.vector.tensor_tensor(wt4[:, 1], wyv1[:], wxv0[:], op=AF.mult)
                nc.vector.tensor_tensor(wt4[:, 2], wyv0[:], wxv1[:], op=AF.mult)
                nc.vector.tensor_tensor(wt4[:, 3], wyv1[:], wxv1[:], op=AF.mult)
                wt4h = pp.tile([128, 4, T], FP16)
                nc.vector.tensor_copy(wt4h[:], wt4[:])
                # weights -> drained layout [(g,r32), j, ls, t, k], g-replicated.
                # lane = ls*32 + r32, so per-ls source partitions are contiguous.
                for g in range(4):
                    for j in range(4):
                        for ls in range(4):
                            nc.sync.dma_start(
                                w4dr[g * 32:(g + 1) * 32, j, ls],
                                wt4h[ls * 32:(ls + 1) * 32, j].rearrange(
                                    "r (t k) -> r t k", k=K))

                # ---- gather indices ----
                # idx = (y0 - rowp)*W + x0 + ((HALO-64)*W - 64), in x-pair units
                idxf = mp.tile([128, T], F32, tag="idxf")
                nc.vector.tensor_tensor(idxf[:].rearrange("p (t k) -> p t k", k=K),
                                        y0f[:].rearrange("p (t k) -> p t k", k=K),
                                        bc_k(rowp[:]), op=AF.subtract)
                nc.vector.tensor_scalar(idxf[:], idxf[:], float(W), float((HALO - 64) * W - 64),
                                        op0=AF.mult, op1=AF.add)
                nc.vector.tensor_tensor(idxf[:], idxf[:], x0f[:], op=AF.add)
                ipix = mp.tile([128, T, 2], I16)
                nc.vector.tensor_copy(ipix[:, :, 0], idxf[:])
                nc.vector.tensor_scalar(ipix[:, :, 1], ipix[:, :, 0], W, None, op0=AF.add)
                nc.vector.tensor_scalar(ipix[:].rearrange("p t y -> p (t y)"),
                                        ipix[:].rearrange("p t y -> p (t y)"), 0, NSRC - 1,
                                        op0=AF.max, op1=AF.min)
                # wrapped idx layout: [p16, (t, k, yc, lh)] replicated x8
                for lh in range(8):
                    nc.sync.dma_start(
                        wrapped[0:16, :, :, :, lh],
                        ipix[lh * 16:(lh + 1) * 16].rearrange("p (t k) y -> p t k y", k=K))
                for cc in range(1, 8):
                    nc.sync.dma_start(wrapped[cc * 16:(cc + 1) * 16], wrapped[0:16])

            # ---- main loop: gather -> left-mult -> group matmuls -> drain ----
            in_ap = AP(right_t, 0, [[C, NSRC], [1, 2 * C]])
            for t in range(NT):
                # gather in 3 calls of 768 idxs (3 k's each); transpose-mode
                # SWDGE calls above ~768 idxs crash the exec unit.
                # stream per t: (k, yc, lh, p16); lane = lh*16+p16 = ls*32+r
                g = gpool.tile([128, 3, 4, NI_T // 3], FP16, tag="g")
                wslice = wrapped[:, t].rearrange("p k y l -> p (k y l)")
                for b in range(3):
                    nc.gpsimd.dma_gather(
                        g[:, b], in_ap,
                        wslice[:, b * 48:(b + 1) * 48],
                        NI_T // 3, NI_T // 3, 2 * C,
                        elem_step=C, transpose=True,
                        queue_num=(3 * t + b) % 4)
                prod = prpool.tile([128, 3, 4, NI_T // 3], FP16, tag="prod")
                for b in range(3):
                    for q in range(4):
                        ch = q % 2
                        lv = left[:, ch].rearrange("c (n l) -> c n l", l=128)[:, t]
                        lv = lv.rearrange("c (lh p) -> c () lh p", p=16)
                        lv = lv.broadcast_to([128, 6, 8, 16])
                        nc.vector.tensor_tensor(
                            prod[:, b, q].rearrange("c (m l p) -> c m l p", m=6, p=16),
                            g[:, b, q].rearrange("c (m l p) -> c m l p", m=6, p=16),
                            lv, op=AF.mult)
                ps = pspool.tile([128, 4 * 4 * K], F32, tag="ps")
                psv = ps[:].rearrange("p (j l k2) -> p j l k2", j=4, l=4)
                for j in range(4):
                    x, yc = j // 2, j % 2
                    for ch in range(2):
                        off = 128 if ch == 0 else 64
                        # lane = ls*32 + r, r = e*16 + p16; lh = ls*2 + e
                        vv = prod[:].rearrange(
                            "c b q (k3 y l2 e p) -> c b q k3 y l2 e p",
                            k3=3, y=2, l2=4, e=2)
                        for r in range(32):
                            e2, p16 = r // 16, r % 16
                            rhs = vv[:, :, x * 2 + ch, :, yc, :, e2, p16].rearrange(
                                "c b k3 l2 -> c l2 b k3")
                            nc.tensor.matmul(
                                psv[:, j], P[:, off - r:off - r + 128], rhs,
                                start=(ch == 0 and r == 0),
                                stop=(ch == 1 and r == 31),
                                skip_group_check=True)
                nc.scalar.copy(drained[:, :, :, t, :],
                               ps[:].rearrange("p (j l k2) -> p j l k2", j=4, l=4))

            # ---- tail: corner weights + 4-corner reduce (j outermost free) ----
            nc.vector.tensor_tensor(
                drained[:].rearrange("p j l t k2 -> p (j l t k2)"),
                drained[:].rearrange("p j l t k2 -> p (j l t k2)"),
                w4dr[:].rearrange("p j l t k2 -> p (j l t k2)"), op=AF.mult)
            nc.vector.tensor_reduce(
                finals[:].rearrange("p l t k2 -> p (l t k2) ()"),
                drained[:].rearrange("p j l t k2 -> p (l t k2) j"),
                axis=mybir.AxisListType.X, op=AF.add)
            nc.sync.dma_start(out_t.ap(), finals[:])
    nc.compile()
    _cache["nc"] = nc
    return nc


def make_in_maps(left_feature, right_feature, flow, extra_offset):
    left_feature = np.asarray(left_feature, dtype=np.float32)
    right_feature = np.asarray(right_feature, dtype=np.float32)
    flow = np.asarray(flow, dtype=np.float32)
    extra_offset = np.asarray(extra_offset, dtype=np.float32)

    pix = np.arange(NPIX)
    lane, tt = pix % 128, pix // 128

    def plane(vals):
        p = np.zeros((128, NT), np.float32)
        p[lane, tt] = vals
        return p

    kgrid = np.arange(-4, 5, dtype=np.float32)
    in_maps = []
    for b in range(B):
        l_cm = (left_feature[b] / GC).astype(np.float16)   # [C, H, W]
        r_cl = np.ascontiguousarray(
            right_feature[b].transpose(1, 2, 0)).astype(np.float16)  # [H, W, C]
        eo = extra_offset[b].reshape(K, 2, H, W)
        for q in range(4):
            row0 = q * HQ
            lo, hi = row0 - HALO, row0 + HQ + HALO
            r_slice = np.zeros((NSRC + 1, C), np.float16)
            clo, chi = max(lo, 0), min(hi, H)
            r_slice[(clo - lo) * W:(chi - lo) * W] = \
                r_cl[clo:chi].reshape((chi - clo) * W, C)
            lq = l_cm[:, row0:row0 + HQ].reshape(C, NPIX)   # pixel = row-major
            # left_cm[h, c, pixel p] with p = t*128 + lane (plane layout)
            l_dev = np.zeros((2, 128, NPIX), np.float16)
            l_dev[:, :, :] = lq.reshape(2, 128, NPIX)
            # permute pixels into (t, lane) order: dev pixel index t*128+lane
            # maps to raster pixel lane? No: plane() stores raster pix p at
            # [p % 128, p // 128]; device pixel (t,lane) = raster t*128+lane
            # only if lane == p % 128 and t == p // 128 -> identity. ok.
            hgl = plane((pix // W + row0).astype(np.float32))
            rwp = np.full((128, NT), np.float32(row0), np.float32)
            wgr = (pix % W).astype(np.float32)
            wkp = np.stack([plane(wgr + kg) for kg in kgrid], axis=1)  # [128,K,NT]
            wkp = np.ascontiguousarray(wkp.transpose(0, 2, 1))        # [128,NT,K]
            fl = np.stack([plane(flow[b, c_][row0:row0 + HQ].ravel()) for c_ in range(2)])
            ex = np.stack([np.stack([plane(eo[k_, c_, row0:row0 + HQ].ravel())
                                     for k_ in range(K)], axis=1) for c_ in range(2)])
            ex = np.ascontiguousarray(ex.transpose(0, 1, 3, 2))       # [2,128,NT,K]
            in_maps.append({
                "right_cl": r_slice,
                "left_cm": l_dev,
                "flow_t": np.ascontiguousarray(fl),
                "extra_t": np.ascontiguousarray(ex),
                "hglob": hgl,
                "wk": wkp,
                "rowp": rwp,
            })
    return in_maps


def assemble(results):
    out = np.zeros((B, G * K, H, W), np.float32)
    for b in range(B):
        for q in range(4):
            o = results[b * 4 + q]["out"]          # [128, 4, NT, K] f32
            # finals[p=(g, r32), ls, t, k] -> pixel = t*128 + ls*32 + r32
            o = o.reshape(G, 32, 4, NT, K)          # [g, r, ls, t, k]
            full = o.transpose(0, 4, 3, 2, 1).reshape(G, K, NPIX)
            out[b, :, q * HQ:(q + 1) * HQ] = full.reshape(G * K, HQ, W)
    return out


def kernel(left_feature, right_feature, flow, extra_offset):
    nc = _build()
    in_maps = make_in_maps(left_feature, right_feature, flow, extra_offset)
    res = bass_utils.run_bass_kernel_spmd(nc, in_maps, list(range(8)))
    return assemble(res.results)


# revision 33
# speedup vs baseline: 2.4206x; 1.1190x over previous
"""CREStereo deformable local correlation on 8 Trainium2 NeuronCores.

Sharding: data-parallel over (batch 2) x (H quarters 4) = 8 cores.

Per core (v2, channel-major matmul pipeline):
  - SWDGE transpose-gather of fp16 x-pair feature elements (2 pixels x 256ch
    = 1KB) into channel-major SBUF tiles: out[c, (x,ch), i].
  - DVE multiplies gathered features by left (channel-major, pre-scaled 1/64)
    in fp16 at 2x rate.
  - TensorE contracts the 64-channel groups with ones-block stationaries
    (r-replica trick spreads 128 pixel-lanes x 4 groups x 4 corners over
    PSUM partitions), accumulating the two 128-channel halves in PSUM.
  - Corner (bilinear) weights apply post-contraction at scalar level on DVE,
    followed by a 4-corner reduce; all index/weight math runs on device.
All value arithmetic happens on device; the host only slices/transposes/pads
(layout) and concatenates shards.
"""
import sys
sys.path.insert(0, "/opt/trn_rl_repo")
import numpy as np

import concourse.bass as bass
import concourse.bacc as bacc
import concourse.mybir as mybir
import concourse.tile as tile
from concourse.ap import AP
from concourse import bass_utils
from concourse.library_config import mlp

B, C, H, W = 2, 256, 96, 192
K, G, GC = 9, 4, 64
HQ = H // 4            # 24 rows per shard
HALO = 12
ROWS = HQ + 2 * HALO   # 48
NPIX = HQ * W          # 4608 pixels per shard
NT = NPIX // 128       # 36 tiles of 128 pixels
NSRC = ROWS * W        # 9216 gatherable pixel rows
T = NT * K             # 324 (t,k) tiles
NI_T = K * 2 * 128     # 2304 gather idxs per t (k, yc, lane)
F32 = mybir.dt.float32
FP16 = mybir.dt.float16
I16 = mybir.dt.int16

_cache = {}


def _build():
    if "nc" in _cache:
        return _cache["nc"]
    nc = bacc.Bacc("TRN2", debug=False, num_devices=8, num_swdge_queues=4)
    right_t = nc.dram_tensor("right_cl", [NSRC + 1, C], FP16, kind="ExternalInput")
    left_t = nc.dram_tensor("left_cm", [2, 128, NPIX], FP16, kind="ExternalInput")
    flow_t = nc.dram_tensor("flow_t", [2, 128, NT], F32, kind="ExternalInput")
    extra_t = nc.dram_tensor("extra_t", [2, 128, NT, K], F32, kind="ExternalInput")
    hglob_t = nc.dram_tensor("hglob", [128, NT], F32, kind="ExternalInput")
    wk_t = nc.dram_tensor("wk", [128, NT, K], F32, kind="ExternalInput")
    rowp_t = nc.dram_tensor("rowp", [128, NT], F32, kind="ExternalInput")
    out_t = nc.dram_tensor("out", [128, 4, NT, K], F32, kind="ExternalOutput")

    AF = mybir.AluOpType
    with tile.TileContext(nc) as tc:
        with tc.tile_pool(name="persist", bufs=1) as pp, \
             tc.tile_pool(name="gather", bufs=2) as gpool, \
             tc.tile_pool(name="prod", bufs=2) as prpool, \
             tc.psum_pool(name="ps", bufs=4) as pspool:
            nc.gpsimd.load_library(mlp)

            # ---- persistent tiles ----
            left = pp.tile([128, 2, NPIX], FP16)
            nc.sync.dma_start(left[:], left_t.ap().rearrange("h c p -> c h p"))
            # ones-pattern for stationary windows
            P = pp.tile([128, 256], FP16)
            nc.vector.memset(P[:], 0)
            nc.vector.memset(P[0:64, 128:129], 1.0)
            nc.vector.memset(P[64:128, 160:161], 1.0)
            NTC = NT // 2
            wrappedA = pp.tile([128, NTC, K, 2, 8], I16)
            wrappedB = pp.tile([128, NTC, K, 2, 8], I16)
            # drained group-dots: [(g, r32), j, ls, t, k], lane = r32*4 + ls
            drained = pp.tile([128, 4, 4, NT, K], FP16)
            w4dr = pp.tile([128, 4, 4, NT, K], FP16)  # [(g,r32), j, ls, t, k]
            finals = pp.tile([128, 4, NT, K], F32)    # [(g,r32), ls, t, k]

            # ---- index & weight math (freed after this block) ----
            with tc.tile_pool(name="math", bufs=1) as mp:
                flow = mp.tile([128, 2, NT], F32)
                nc.sync.dma_start(flow[:], flow_t.ap().rearrange("c p t -> p c t"))
                extra = mp.tile([128, 2, T], F32)
                nc.sync.dma_start(extra[:], extra_t.ap().rearrange("c p t k -> p c (t k)"))
                hglob = mp.tile([128, NT], F32)
                nc.sync.dma_start(hglob[:], hglob_t.ap())
                wk = mp.tile([128, T], F32)
                nc.sync.dma_start(wk[:], wk_t.ap().rearrange("p t k -> p (t k)"))
                rowp = mp.tile([128, NT], F32)
                nc.sync.dma_start(rowp[:], rowp_t.ap())

                def bc_k(ap2d):  # [128, NT] -> [128, NT, K(bcast)]
                    return ap2d.rearrange("p (t o) -> p t o", o=1).broadcast_to([128, NT, K])

                xq = mp.tile([128, NT, K], F32)
                yq = mp.tile([128, NT, K], F32)
                nc.vector.tensor_tensor(xq[:], extra[:, 0].rearrange("p (t k) -> p t k", k=K),
                                        bc_k(flow[:, 0]), op=AF.add)
                nc.vector.tensor_tensor(xq[:], xq[:], wk[:].rearrange("p (t k) -> p t k", k=K),
                                        op=AF.add)
                nc.gpsimd.tensor_tensor(yq[:], extra[:, 1].rearrange("p (t k) -> p t k", k=K),
                                        bc_k(flow[:, 1]), op=AF.add)
                nc.gpsimd.tensor_tensor(yq[:], yq[:], bc_k(hglob[:]), op=AF.add)
                # biased coords (positive -> trunc == floor)
                xb = mp.tile([128, T], F32)
                yb = mp.tile([128, T], F32)
                nc.vector.tensor_scalar(xb[:], xq[:].rearrange("p t k -> p (t k)"), 64.0, None, op0=AF.add)
                nc.gpsimd.tensor_scalar(yb[:], yq[:].rearrange("p t k -> p (t k)"), 64.0, None, op0=AF.add)
                x0i = mp.tile([128, T], I16)
                y0i = mp.tile([128, T], I16)
                nc.vector.tensor_copy(x0i[:], xb[:])
                nc.vector.tensor_copy(y0i[:], yb[:])
                x0f = mp.tile([128, T], F32)
                y0f = mp.tile([128, T], F32)
                nc.vector.tensor_copy(x0f[:], x0i[:])
                nc.vector.tensor_copy(y0f[:], y0i[:])
                # cast rounding mode differs sim vs hw; force floor: t -= (t > x)
                gfix = mp.tile([128, T], F32, tag="gfix")
                nc.vector.tensor_tensor(gfix[:], x0f[:], xb[:], op=AF.is_gt)
                nc.vector.tensor_tensor(x0f[:], x0f[:], gfix[:], op=AF.subtract)
                gfy = mp.tile([128, T], F32, tag="gfy")
                nc.vector.tensor_tensor(gfy[:], y0f[:], yb[:], op=AF.is_gt)
                nc.gpsimd.tensor_tensor(y0f[:], y0f[:], gfy[:], op=AF.subtract)
                wx1 = mp.tile([128, T], F32)
                wy1 = mp.tile([128, T], F32)
                nc.vector.tensor_tensor(wx1[:], xb[:], x0f[:], op=AF.subtract)
                nc.gpsimd.tensor_tensor(wy1[:], yb[:], y0f[:], op=AF.subtract)
                wx0 = mp.tile([128, T], F32)
                wy0 = mp.tile([128, T], F32)
                # 1 - w = (w - 1) * -1
                nc.vector.tensor_scalar(wx0[:], wx1[:], 1.0, -1.0, op0=AF.subtract, op1=AF.mult)
                nc.gpsimd.tensor_scalar(wy0[:], wy1[:], 1.0, -1.0, op0=AF.subtract, op1=AF.mult)

                def valid(dst, src, lo, hi):
                    t1 = mp.tile([128, T], F32, tag="vtmp1")
                    nc.vector.tensor_scalar(t1[:], src[:], float(lo), None, op0=AF.is_ge)
                    t2 = mp.tile([128, T], F32, tag="vtmp2")
                    nc.vector.tensor_scalar(t2[:], src[:], float(hi), None, op0=AF.is_le)
                    nc.vector.tensor_tensor(dst[:], t1[:], t2[:], op=AF.mult)
                vx0 = mp.tile([128, T], F32)
                vx1 = mp.tile([128, T], F32)
                vy0 = mp.tile([128, T], F32)
                vy1 = mp.tile([128, T], F32)
                valid(vx0, x0f, 64, 64 + W - 1)
                valid(vx1, x0f, 63, 64 + W - 2)
                valid(vy0, y0f, 64, 64 + H - 1)
                valid(vy1, y0f, 63, 64 + H - 2)
                wxv0, wxv1 = vx0, vx1   # reuse in place
                nc.vector.tensor_tensor(wxv0[:], wx0[:], vx0[:], op=AF.mult)
                nc.vector.tensor_tensor(wxv1[:], wx1[:], vx1[:], op=AF.mult)
                wyv0, wyv1 = vy0, vy1
                nc.vector.tensor_tensor(wyv0[:], wy0[:], vy0[:], op=AF.mult)
                nc.vector.tensor_tensor(wyv1[:], wy1[:], vy1[:], op=AF.mult)
                # corner weights, j = x*2 + yc
                wt4 = mp.tile([128, 4, T], F32)
                nc.vector.tensor_tensor(wt4[:, 0], wyv0[:], wxv0[:], op=AF.mult)
                nc.vector.tensor_tensor(wt4[:, 1], wyv1[:], wxv0[:], op=AF.mult)
                nc.vector.tensor_tensor(wt4[:, 2], wyv0[:], wxv1[:], op=AF.mult)
                nc.vector.tensor_tensor(wt4[:, 3], wyv1[:], wxv1[:], op=AF.mult)
                wt4h = pp.tile([128, 4, T], FP16)
                nc.vector.tensor_copy(wt4h[:], wt4[:])
                # weights -> drained layout [(g,r32), j, ls, t, k], g-replicated.
                # lane = ls*32 + r32, so per-ls source partitions are contiguous.
                qeng = [nc.sync, nc.scalar]
                for j in range(4):
                    for ls in range(4):
                        qeng[(j * 4 + ls) % 2].dma_start(
                            w4dr[0:32, j, ls],
                            wt4h[ls * 32:(ls + 1) * 32, j].rearrange(
                                "r (t k) -> r t k", k=K))
                nc.scalar.dma_start(w4dr[32:64], w4dr[0:32])
                nc.sync.dma_start(w4dr[64:128], w4dr[0:64])

                # ---- gather indices ----
                # idx = (y0 - rowp)*W + x0 + ((HALO-64)*W - 64), in x-pair units
                idxf = mp.tile([128, T], F32, tag="idxf")
                nc.vector.tensor_tensor(idxf[:].rearrange("p (t k) -> p t k", k=K),
                                        y0f[:].rearrange("p (t k) -> p t k", k=K),
                                        bc_k(rowp[:]), op=AF.subtract)
                nc.vector.tensor_scalar(idxf[:], idxf[:], float(W), float((HALO - 64) * W - 64),
                                        op0=AF.mult, op1=AF.add)
                nc.vector.tensor_tensor(idxf[:], idxf[:], x0f[:], op=AF.add)
                ipix = mp.tile([128, T, 2], I16)
                nc.vector.tensor_copy(ipix[:, :, 0], idxf[:])
                nc.vector.tensor_scalar(ipix[:, :, 1], ipix[:, :, 0], W, None, op0=AF.add)
                nc.vector.tensor_scalar(ipix[:].rearrange("p t y -> p (t y)"),
                                        ipix[:].rearrange("p t y -> p (t y)"), 0, NSRC - 1,
                                        op0=AF.max, op1=AF.min)
                # wrapped idx layout: [p16, (t, k, yc, lh)] replicated x8,
                # built in 2 t-chunks so gathers can start after chunk A.
                ipv = ipix[:].rearrange("p (t k) y -> p t k y", k=K)
                for ci, wr in enumerate((wrappedA, wrappedB)):
                    for lh in range(8):
                        qeng[lh % 2].dma_start(
                            wr[0:16, :, :, :, lh],
                            ipv[lh * 16:(lh + 1) * 16, ci * NTC:(ci + 1) * NTC])
                    nc.scalar.dma_start(wr[16:32], wr[0:16])
                    nc.sync.dma_start(wr[32:64], wr[0:32])
                    nc.sync.dma_start(wr[64:128], wr[0:64])

            # ---- main loop: gather -> left-mult -> group matmuls -> drain ----
            in_ap = AP(right_t, 0, [[C, NSRC], [1, 2 * C]])
            for t in range(NT):
                # gather in 3 calls of 768 idxs (3 k's each); transpose-mode
                # SWDGE calls above ~768 idxs crash the exec unit.
                # stream per t: (k, yc, lh, p16); lane = lh*16+p16 = ls*32+r
                g = gpool.tile([128, 3, 4, NI_T // 3], FP16, tag="g")
                wr = wrappedA if t < NTC else wrappedB
                wslice = wr[:, t % NTC].rearrange("p k y l -> p (k y l)")
                for b in range(3):
                    nc.gpsimd.dma_gather(
                        g[:, b], in_ap,
                        wslice[:, b * 48:(b + 1) * 48],
                        NI_T // 3, NI_T // 3, 2 * C,
                        elem_step=C, transpose=True,
                        queue_num=(3 * t + b) % 4)
                prod = prpool.tile([128, 3, 4, NI_T // 3], FP16, tag="prod")
                for b in range(3):
                    for q in range(4):
                        ch = q % 2
                        lv = left[:, ch].rearrange("c (n l) -> c n l", l=128)[:, t]
                        lv = lv.rearrange("c (lh p) -> c () lh p", p=16)
                        lv = lv.broadcast_to([128, 6, 8, 16])
                        nc.vector.tensor_tensor(
                            prod[:, b, q].rearrange("c (m l p) -> c m l p", m=6, p=16),
                            g[:, b, q].rearrange("c (m l p) -> c m l p", m=6, p=16),
                            lv, op=AF.mult)
                ps = pspool.tile([128, 4 * 4 * K], F32, tag="ps")
                psv = ps[:].rearrange("p (j l k2) -> p j l k2", j=4, l=4)
                for j in range(4):
                    x, yc = j // 2, j % 2
                    for ch in range(2):
                        off = 128 if ch == 0 else 64
                        # lane = ls*32 + r, r = e*16 + p16; lh = ls*2 + e
                        vv = prod[:].rearrange(
                            "c b q (k3 y l2 e p) -> c b q k3 y l2 e p",
                            k3=3, y=2, l2=4, e=2)
                        for r in range(32):
                            e2, p16 = r // 16, r % 16
                            rhs = vv[:, :, x * 2 + ch, :, yc, :, e2, p16].rearrange(
                                "c b k3 l2 -> c l2 b k3")
                            nc.tensor.matmul(
                                psv[:, j], P[:, off - r:off - r + 128], rhs,
                                start=(ch == 0 and r == 0),
                                stop=(ch == 1 and r == 31),
                                skip_group_check=True)
                nc.scalar.copy(drained[:, :, :, t, :],
                               ps[:].rearrange("p (j l k2) -> p j l k2", j=4, l=4))

            # ---- tail: corner weights + 4-corner reduce (j outermost free) ----
            nc.vector.tensor_tensor(
                drained[:].rearrange("p j l t k2 -> p (j l t k2)"),
                drained[:].rearrange("p j l t k2 -> p (j l t k2)"),
                w4dr[:].rearrange("p j l t k2 -> p (j l t k2)"), op=AF.mult)
            nc.vector.tensor_reduce(
                finals[:].rearrange("p l t k2 -> p (l t k2) ()"),
                drained[:].rearrange("p j l t k2 -> p (l t k2) j"),
                axis=mybir.AxisListType.X, op=AF.add)
            nc.sync.dma_start(out_t.ap(), finals[:])
    nc.compile()
    _cache["nc"] = nc
    return nc


def make_in_maps(left_feature, right_feature, flow, extra_offset):
    left_feature = np.asarray(left_feature, dtype=np.float32)
    right_feature = np.asarray(right_feature, dtype=np.float32)
    flow = np.asarray(flow, dtype=np.float32)
    extra_offset = np.asarray(extra_offset, dtype=np.float32)

    pix = np.arange(NPIX)
    lane, tt = pix % 128, pix // 128

    def plane(vals):
        p = np.zeros((128, NT), np.float32)
        p[lane, tt] = vals
        return p

    kgrid = np.arange(-4, 5, dtype=np.float32)
    in_maps = []
    for b in range(B):
        l_cm = (left_feature[b] / GC).astype(np.float16)   # [C, H, W]
        r_cl = np.ascontiguousarray(
            right_feature[b].transpose(1, 2, 0)).astype(np.float16)  # [H, W, C]
        eo = extra_offset[b].reshape(K, 2, H, W)
        for q in range(4):
            row0 = q * HQ
            lo, hi = row0 - HALO, row0 + HQ + HALO
            r_slice = np.zeros((NSRC + 1, C), np.float16)
            clo, chi = max(lo, 0), min(hi, H)
            r_slice[(clo - lo) * W:(chi - lo) * W] = \
                r_cl[clo:chi].reshape((chi - clo) * W, C)
            lq = l_cm[:, row0:row0 + HQ].reshape(C, NPIX)   # pixel = row-major
            # left_cm[h, c, pixel p] with p = t*128 + lane (plane layout)
            l_dev = np.zeros((2, 128, NPIX), np.float16)
            l_dev[:, :, :] = lq.reshape(2, 128, NPIX)
            # permute pixels into (t, lane) order: dev pixel index t*128+lane
            # maps to raster pixel lane? No: plane() stores raster pix p at
            # [p % 128, p // 128]; device pixel (t,lane) = raster t*128+lane
            # only if lane == p % 128 and t == p // 128 -> identity. ok.
            hgl = plane((pix // W + row0).astype(np.float32))
            rwp = np.full((128, NT), np.float32(row0), np.float32)
            wgr = (pix % W).astype(np.float32)
            wkp = np.stack([plane(wgr + kg) for kg in kgrid], axis=1)  # [128,K,NT]
            wkp = np.ascontiguousarray(wkp.transpose(0, 2, 1))        # [128,NT,K]
            fl = np.stack([plane(flow[b, c_][row0:row0 + HQ].ravel()) for c_ in range(2)])
            ex = np.stack([np.stack([plane(eo[k_, c_, row0:row0 + HQ].ravel())
                                     for k_ in range(K)], axis=1) for c_ in range(2)])
            ex = np.ascontiguousarray(ex.transpose(0, 1, 3, 2))       # [2,128,NT,K]
            in_maps.append({
                "right_cl": r_slice,
                "left_cm": l_dev,
                "flow_t": np.ascontiguousarray(fl),
                "extra_t": np.ascontiguousarray(ex),
                "hglob": hgl,
                "wk": wkp,
                "rowp": rwp,
            })
    return in_maps


def assemble(results):
    out = np.zeros((B, G * K, H, W), np.float32)
    for b in range(B):
        for q in range(4):
            o = results[b * 4 + q]["out"]          # [128, 4, NT, K] f32
            # finals[p=(g, r32), ls, t, k] -> pixel = t*128 + ls*32 + r32
            o = o.reshape(G, 32, 4, NT, K)          # [g, r, ls, t, k]
            full = o.transpose(0, 4, 3, 2, 1).reshape(G, K, NPIX)
            out[b, :, q * HQ:(q + 1) * HQ] = full.reshape(G * K, HQ, W)
    return out


def kernel(left_feature, right_feature, flow, extra_offset):
    nc = _build()
    in_maps = make_in_maps(left_feature, right_feature, flow, extra_offset)
    res = bass_utils.run_bass_kernel_spmd(nc, in_maps, list(range(8)))
    return assemble(res.results)
